# revision 5
# baseline (speedup 1.0000x reference)
"""DynamicGate MoE routing kernel for Trainium2 (8 NeuronCores, Bass/Tile).

Computes, for x[N,H], sim_matrix[H,E], gates[E]:
    logits = l2norm_rows(x) @ l2norm_cols(sim_matrix)
    thr    = sigmoid(gates)
    gated  = relu(logits - thr)
    mask   = (gated > 0), with top-1 fallback for all-inactive tokens
    probs  = softmax over active experts of gated
Returns (mask, probs, logits), all [N, E] fp32.

Sharding: data-parallel on the token dim across 8 cores (2048 tokens per
core); sim_matrix/gates replicated. No collectives needed.

Strategy (v2):
  - x is pre-rounded to FP32R (fp32 with 11 explicit mantissa bits) on the
    host - a bitwise no-op for DMA, and lets the PE run matmuls at 1
    cycle/row (4x the plain-fp32 rate) and transposes at 1.5 cycles/row.
  - logits are computed TRANSPOSED: for each 512-token tile,
    plg[64,512] += wn_c^T @ xt_c over 16 h-chunks, with the tiny wn as the
    stationary operand (64-column weight loads) and tokens as the wide
    moving operand.
  - per-token sum-of-squares runs as fused square+accumulate, split
    between ACT (activation Square accum_out) and DVE (stt accum_out).
  - PSUM->SBUF copies of transposed x are split between ACT and DVE.
  - epilogue: transpose logits^T back to [tok, E] blocks (fp32, exact),
    then mask/probs with argmax comparisons on full-fp32 values; bf16
    elementwise where precision allows; bf16 DMA-out, upcast on host.
"""

import sys

if "/opt/trn_rl_repo" not in sys.path:
    sys.path.insert(0, "/opt/trn_rl_repo")

import numpy as np

import concourse.bacc as bacc
import concourse.mybir as mybir
from concourse import bass_utils, masks
from concourse.tile import TileContext

F32 = mybir.dt.float32
F32R = mybir.dt.float32r
BF16 = mybir.dt.bfloat16
OP = mybir.AluOpType
AF = mybir.ActivationFunctionType
AX = mybir.AxisListType

N, H, E = 16384, 2048, 64
NCORES = 8
NLOC = N // NCORES     # 2048 tokens per core
PB = 128               # tokens per block (partition dim)
HC = H // 128          # 16 h-chunks
TB = 512               # tokens per tile
NBLK = TB // PB        # 4 blocks per tile
NTILE = NLOC // TB     # 4 tiles per core
EPS = 1e-12

# per-tile engine split knobs
SUMSQ_ON_DVE = 1       # of NBLK sumsq blocks, how many go to DVE (rest ACT)
COPIES_ON_DVE = 4      # of 8 xt copies per tile, how many go to DVE (rest ACT)


def build():
    nc = bacc.Bacc("TRN2", target_bir_lowering=False, debug=False)
    x_d = nc.dram_tensor("x", [NLOC, H], F32R, kind="ExternalInput")
    sim_d = nc.dram_tensor("sim", [H, E], F32, kind="ExternalInput")
    gates_d = nc.dram_tensor("gates", [1, E], F32, kind="ExternalInput")
    mask_d = nc.dram_tensor("mask", [NLOC, E], BF16, kind="ExternalOutput")
    probs_d = nc.dram_tensor("probs", [NLOC, E], BF16, kind="ExternalOutput")
    logits_d = nc.dram_tensor("logits", [NLOC, E], BF16, kind="ExternalOutput")

    with TileContext(nc) as tc:
        with (
            tc.tile_pool(name="const", bufs=1) as constp,
            tc.tile_pool(name="xin", bufs=6) as xinp,
            tc.tile_pool(name="xt", bufs=2) as xtp,
            tc.tile_pool(name="sq", bufs=1) as sqp,
            tc.tile_pool(name="ep", bufs=2) as epp,
            tc.tile_pool(name="sc", bufs=2) as scp,
            tc.tile_pool(name="psT", bufs=2, space="PSUM") as psT,
            tc.tile_pool(name="psL", bufs=2, space="PSUM") as psL,
            tc.tile_pool(name="psB", bufs=2, space="PSUM") as psB,
        ):
            # ---- constants -----------------------------------------------
            ident_f = constp.tile([128, 128], F32, name="ident_f")
            masks.make_identity(nc, ident_f)
            ident_r = constp.tile([128, 128], F32R, name="ident_r")
            nc.vector.tensor_copy(ident_r, ident_f)
            onesc = constp.tile([128, 1], F32, name="onesc")
            nc.gpsimd.memset(onesc, 1.0)
            onesr = constp.tile([1, 128], F32, name="onesr")
            nc.gpsimd.memset(onesr, 1.0)

            wn = constp.tile([128, HC * E], F32, name="wn")
            g_row = constp.tile([1, E], F32, name="g_row")

            def emit_const_dmas():
                nc.sync.dma_start(
                    out=wn.rearrange("p (c e) -> p c e", e=E),
                    in_=sim_d.ap().rearrange("(c p) e -> p c e", p=128),
                )
                nc.sync.dma_start(out=g_row, in_=gates_d.ap())

            # wn_s: column-normalized sim, f32r, chunk-major [128, c, e]
            wn_s = constp.tile([128, HC, E], F32R, name="wn_s")
            thr_bb = constp.tile([128, E], BF16, name="thr_bb")

            def emit_wn_preamble():
                wnsq = constp.tile([128, HC * E], F32, name="wnsq")
                nc.scalar.square(wnsq, wn)
                csb = psB.tile([128, NBLK, E], F32, name="csb", tag="ptb")
                cs_ps = csb[0:1, 0, :]
                for c in range(HC):
                    nc.tensor.matmul(
                        cs_ps, lhsT=onesc, rhs=wnsq[:, c * E:(c + 1) * E],
                        start=(c == 0), stop=(c == HC - 1),
                    )
                # rwn = 1/max(sqrt(cs), EPS) = exp(-0.5*ln(max(cs, EPS^2)))
                csm = constp.tile([1, E], F32, name="csm")
                nc.vector.tensor_scalar(
                    out=csm, in0=cs_ps, scalar1=EPS * EPS, scalar2=None,
                    op0=OP.max,
                )
                lncs = constp.tile([1, E], F32, name="lncs")
                nc.scalar.activation(lncs, csm, AF.Ln)
                rwn = constp.tile([1, E], F32, name="rwn")
                nc.scalar.activation(rwn, lncs, AF.Exp, scale=-0.5)

                # thr = sigmoid(g) = 1/(1+exp(-g))  (stays in the exp/ln set)
                eneg = constp.tile([1, E], F32, name="eneg")
                nc.scalar.activation(eneg, g_row, AF.Exp, scale=-1.0)
                nc.vector.tensor_scalar(
                    out=eneg, in0=eneg, scalar1=1.0, scalar2=None, op0=OP.add
                )
                thr_row = constp.tile([1, E], F32, name="thr_row")
                nc.vector.reciprocal(thr_row, eneg)

                # broadcast [1,E] rows to 128 partitions via rank-1 matmul
                bcb = psB.tile([128, NBLK, E], F32, name="bcb", tag="ptb")
                bc_ps = bcb.rearrange("p j e -> p (j e)")[:, 0:2 * E]
                nc.tensor.matmul(bc_ps[:, 0:E], lhsT=onesr, rhs=rwn,
                                 start=True, stop=True)
                nc.tensor.matmul(bc_ps[:, E:2 * E], lhsT=onesr, rhs=thr_row,
                                 start=True, stop=True)
                rwn_b = constp.tile([128, E], F32, name="rwn_b")
                nc.scalar.copy(rwn_b, bc_ps[:, 0:E])
                nc.scalar.copy(thr_bb, bc_ps[:, E:2 * E])

                # wn_s[p, c, e] = wn[p, c*E+e] * rwn_b[p, e]  (f32r rounded)
                nc.vector.tensor_tensor(
                    out=wn_s,
                    in0=wn.rearrange("p (c e) -> p c e", e=E),
                    in1=rwn_b.unsqueeze(1).broadcast_to([128, HC, E]),
                    op=OP.mult,
                )

            # ---- main loop: tiles of 512 tokens --------------------------
            x_tiles = {}

            def prefetch(b):
                t = xinp.tile([128, H], F32R, name="x_nat", tag="x_nat")
                nc.sync.dma_start(out=t, in_=x_d.ap()[b * PB:(b + 1) * PB, :])
                x_tiles[b] = t

            for b in range(4):
                prefetch(b)
            emit_const_dmas()
            emit_wn_preamble()

            for ti in range(NTILE):
                blocks = [x_tiles.pop(ti * NBLK + j) for j in range(NBLK)]
                for b in range(4):
                    nb = (ti + 1) * NBLK + b
                    if nb < NLOC // PB:
                        prefetch(nb)

                # -- sumsq per block (fused square+accumulate) -------------
                ssq = scp.tile([128, NBLK], F32, name="ssq", tag="ssq")
                for j in range(NBLK):
                    if j < SUMSQ_ON_DVE:
                        sq = sqp.tile([128, H], F32, name="sqd", tag="sqd")
                        nc.vector.scalar_tensor_tensor(
                            out=sq, in0=blocks[j], scalar=1.0, in1=blocks[j],
                            op0=OP.mult, op1=OP.mult,
                            accum_out=ssq[:, j:j + 1],
                        )
                    else:
                        sq = sqp.tile([128, H], F32, name="sqa", tag="sqa")
                        nc.scalar.activation(
                            sq, blocks[j], AF.Square,
                            accum_out=ssq[:, j:j + 1],
                        )
                # rx = 1/max(sqrt(ssq), eps) = exp(-0.5*ln(max(ssq, eps^2)))
                ssqm = scp.tile([128, NBLK], F32, name="ssqm", tag="ssqm")
                nc.vector.tensor_scalar(
                    out=ssqm, in0=ssq, scalar1=EPS * EPS, scalar2=None,
                    op0=OP.max,
                )
                lnss = scp.tile([128, NBLK], F32, name="lnss", tag="lnss")
                nc.scalar.activation(lnss, ssqm, AF.Ln)
                rx = scp.tile([128, NBLK], F32, name="rx", tag="rx")
                nc.scalar.activation(rx, lnss, AF.Exp, scale=-0.5)

                # -- transpose x: 8 rounds of (2 chunks x 4 blocks) --------
                # xt[p=h, c, t] with t spanning the tile's 512 tokens
                xt = xtp.tile([128, HC, TB], F32R, name="xt", tag="xt")
                for cp in range(HC // 2):
                    pt = psT.tile([128, 2, NBLK, 128], F32R, name="pt", tag="pt")
                    for k in range(2):
                        c = 2 * cp + k
                        for j in range(NBLK):
                            nc.tensor.transpose(
                                pt[:, k, j, :],
                                blocks[j][:, c * 128:(c + 1) * 128],
                                ident_r,
                            )
                    dst = xt[:, 2 * cp:2 * cp + 2, :]
                    src = pt.rearrange("p k j t -> p k (j t)")
                    if (ti + cp) % 3 != 2:
                        nc.vector.tensor_copy(dst, src)
                    else:
                        nc.scalar.copy(dst, src)

                # -- logits^T accumulation: plg[64, 512] -------------------
                plg = psL.tile([64, TB], F32, name="plg", tag="plg")
                for c in range(HC):
                    nc.tensor.matmul(
                        plg, lhsT=wn_s[:, c, :], rhs=xt[:, c, :],
                        start=(c == 0), stop=(c == HC - 1),
                    )

                # -- transpose back to [tok, E] blocks (full fp32) ---------
                lgT = epp.tile([64, TB], F32, name="lgT", tag="lgT")
                nc.scalar.copy(lgT, plg)
                ptb = psB.tile([128, NBLK, E], F32, name="ptb", tag="ptb")
                for j in range(NBLK):
                    nc.tensor.transpose(
                        ptb[:, j, :], lgT[:, j * 128:(j + 1) * 128],
                        ident_f[0:64, 0:64],
                    )

                # -- epilogue on [128, NBLK, E] -----------------------------
                def bce(ap):   # [128, NBLK] -> [128, NBLK, E] stride-0
                    return ap.unsqueeze(2).broadcast_to([128, NBLK, E])

                lmax = scp.tile([128, NBLK], F32, name="lmax", tag="lmax")
                nc.vector.tensor_reduce(
                    out=lmax, in_=ptb, axis=AX.X, op=OP.max,
                )
                onehot = epp.tile([128, NBLK, E], BF16, name="onehot", tag="onehot")
                nc.vector.tensor_tensor(
                    out=onehot, in0=ptb, in1=bce(lmax), op=OP.is_equal,
                )
                logits_bf = epp.tile([128, NBLK, E], BF16, name="logits_bf",
                                     tag="logits_bf")
                nc.vector.tensor_tensor(
                    out=logits_bf, in0=ptb, in1=bce(rx), op=OP.mult,
                )
                gsub = epp.tile([128, NBLK, E], BF16, name="gsub", tag="gsub")
                nc.vector.tensor_tensor(
                    out=gsub, in0=logits_bf,
                    in1=thr_bb.unsqueeze(1).broadcast_to([128, NBLK, E]),
                    op=OP.subtract,
                )
                ind = epp.tile([128, NBLK, E], BF16, name="ind", tag="ind")
                nc.vector.tensor_scalar(
                    out=ind, in0=gsub, scalar1=0.0, scalar2=None, op0=OP.is_gt,
                )
                nact = scp.tile([128, NBLK], F32, name="nact", tag="nact")
                nc.vector.tensor_reduce(
                    out=nact, in_=ind, axis=AX.X, op=OP.add,
                )
                inact = scp.tile([128, NBLK], F32, name="inact", tag="inact")
                nc.vector.tensor_scalar(
                    out=inact, in0=nact, scalar1=0.0, scalar2=None,
                    op0=OP.is_equal,
                )
                maskt = epp.tile([128, NBLK, E], BF16, name="maskt", tag="maskt")
                nc.vector.tensor_tensor(
                    out=maskt, in0=onehot, in1=bce(inact), op=OP.mult,
                )
                nc.vector.tensor_tensor(
                    out=maskt, in0=maskt, in1=ind, op=OP.add,
                )
                # probs = mask*exp(gsub) / sum(mask*exp(gsub))  (gmax-free:
                # gsub <= 1-thr is small, and fallback rows renormalize to 1)
                ex = epp.tile([128, NBLK, E], BF16, name="ex", tag="ex")
                nc.scalar.activation(ex, gsub, AF.Exp)
                me = epp.tile([128, NBLK, E], BF16, name="me", tag="me")
                nc.vector.tensor_tensor(
                    out=me, in0=ex, in1=maskt, op=OP.mult,
                )
                sesum = scp.tile([128, NBLK], F32, name="sesum", tag="sesum")
                nc.vector.tensor_reduce(
                    out=sesum, in_=me, axis=AX.X, op=OP.add,
                )
                rs = scp.tile([128, NBLK], F32, name="rs", tag="rs")
                nc.vector.reciprocal(rs, sesum)
                probs = epp.tile([128, NBLK, E], BF16, name="probs", tag="probs")
                nc.vector.tensor_tensor(
                    out=probs, in0=me, in1=bce(rs), op=OP.mult,
                )

                gtok = slice(ti * TB, (ti + 1) * TB)
                for out_d, src in ((mask_d, maskt), (probs_d, probs),
                                   (logits_d, logits_bf)):
                    nc.gpsimd.dma_start(
                        out=out_d.ap()[gtok, :].rearrange(
                            "(j p) e -> p j e", p=128),
                        in_=src,
                    )

    nc.compile()
    return nc


_NC_CACHE = {}


def _get_nc():
    if "nc" not in _NC_CACHE:
        _NC_CACHE["nc"] = build()
    return _NC_CACHE["nc"]


def _round_f32r(a):
    """Round fp32 to FP32R (11 explicit mantissa bits), nearest-even."""
    b = np.ascontiguousarray(a, dtype=np.float32).view(np.uint32)
    hi = b >> np.uint32(12)
    low = b & np.uint32(0xFFF)
    rnd = (low > np.uint32(0x800)) | (
        (low == np.uint32(0x800)) & ((hi & np.uint32(1)) == np.uint32(1))
    )
    out = (hi + rnd.astype(np.uint32)) << np.uint32(12)
    return out.view(np.float32)


def make_in_maps(x, sim_matrix, gates):
    x = _round_f32r(np.asarray(x, dtype=np.float32))
    sim = np.ascontiguousarray(np.asarray(sim_matrix, dtype=np.float32))
    g = np.ascontiguousarray(np.asarray(gates, dtype=np.float32)).reshape(1, E)
    return [
        {"x": x[c * NLOC:(c + 1) * NLOC], "sim": sim, "gates": g}
        for c in range(NCORES)
    ]


def kernel(x, sim_matrix, gates):
    nc = _get_nc()
    in_maps = make_in_maps(x, sim_matrix, gates)
    res = bass_utils.run_bass_kernel_spmd(nc, in_maps, core_ids=list(range(NCORES)))
    outs = []
    for name in ("mask", "probs", "logits"):
        outs.append(np.concatenate(
            [np.asarray(res.results[c][name], dtype=np.float32)
             for c in range(NCORES)], axis=0))
    return tuple(outs)
